# revision 35
# baseline (speedup 1.0000x reference)
"""Trainium2 Bass kernel for nn_A100GraphBuilder (per-sample community kNN graphs).

Strategy (pure data parallelism, one sample per NeuronCore, B=8 -> 8 cores):
  host prep : per sample, sort nodes by x into 8 communities of 512 (argsort),
              build the K=5 matmul operands that evaluate
                d2[u,v] = sq_u + sq_v - 2*x_u*x_v - 2*y_u*y_v + 1e-30
              as a single PE matmul per [128 x 512] tile.
  device    : per community (512x512 block, 4 row tiles):
                PE   : K=5 fp32 matmul -> d2 tile in PSUM
                DVE  : pack keys  K = (bits(d2) & ~0x1FF) | 0x80000000 | col
                       (negated d2 quantized to 14 mantissa bits with the
                        column index embedded in the low 9 bits -> unique keys,
                        f32-descending order == (d2 asc, col asc))
                DVE  : MAX8 -> top-8 keys; MATCH_REPLACE knocks them out;
                       MAX8 again -> candidates 9..16
              out: [4096, 16] candidate keys per core.
  host post : decode candidate columns, drop the diagonal, re-rank the 16
              candidates with bitwise-exact reference distances (the XLA CPU
              recipe: fma(y*y', f32(x*x'))), take top-8 with (dist, col)
              ties -> exact lax.top_k semantics; weights = 10*exp(-2*d).
              Random inter-community edges replicated with the vmapped jax
              PRNG on CPU.  Concatenate to the reference edge layout.
"""

import os
import sys

import numpy as np

for _p in ("/opt/trn_rl_repo", "/root/.axon_site/_ro/trn_rl_repo"):
    if os.path.isdir(_p) and _p not in sys.path:
        sys.path.append(_p)

import ml_dtypes

import concourse.bass as bass
import concourse.mybir as mybir
from concourse.bass_utils import run_bass_kernel_spmd

B = 8
N = 4096
N_COMM = 8
C = 512
K_NN = 8
N_INTER = 8
BIG = np.float32(1.0e6)
KDIM = 19         # matmul contraction rows (3-way bf16 split, see _host_prep)
NTILE = C // 128  # row tiles per community


# --------------------------------------------------------------------------
# host: exact reference distance recipe (bitwise-identical to XLA CPU)
# --------------------------------------------------------------------------
def _exact_dist(co, sq, u, v):
    """dist[u,v] exactly as the CPU reference computes it (f32 bitwise)."""
    t32 = np.float32(co[u, 0] * co[v, 0])
    g = (
        co[u, 1].astype(np.float64) * co[v, 1].astype(np.float64)
        + t32.astype(np.float64)
    ).astype(np.float32)
    d2 = np.maximum((sq[u] + sq[v]) - np.float32(2.0) * g, np.float32(1e-12))
    return np.sqrt(d2)


def _split3(v):
    """3-way bf16 split: v ~= a + b + c with ~24 mantissa bits total."""
    a = v.astype(ml_dtypes.bfloat16).astype(np.float32)
    b = (v - a).astype(ml_dtypes.bfloat16).astype(np.float32)
    c = (v - a - b).astype(ml_dtypes.bfloat16).astype(np.float32)
    return a, b, c


def _host_prep(coords):
    """comm_nodes [B,NC,C] and packed bf16 matmul operands UV [B,19,2*NC*C].

    d2 = sq_u + sq_v - 2 x_u x_v - 2 y_u y_v + 1e-30 evaluated as a K=19
    bf16 matmul: each f32 factor is split into 3 bf16 terms and the
    significant cross-products are accumulated in fp32 PSUM (error ~1e-5,
    well inside the candidate scheme's tolerance).  Columns 0..NC*C hold
    the stationary operand rows, columns NC*C..2*NC*C the moving rows.
    """
    comm_nodes = np.empty((B, N_COMM, C), dtype=np.int64)
    UV = np.zeros((B, KDIM, 2 * N_COMM * C), dtype=ml_dtypes.bfloat16)
    for b in range(B):
        x = coords[b, :, 0]
        y = coords[b, :, 1]
        sq = x * x + y * y
        order = np.argsort(x, kind="stable")
        cn = np.sort(order.reshape(N_COMM, C), axis=1)
        comm_nodes[b] = cn
        flat = cn.reshape(-1)
        cx, cy, csq = x[flat], y[flat], sq[flat]
        s1, s2, s3 = _split3(csq)
        x1, x2, x3 = _split3(cx)
        y1, y2, y3 = _split3(cy)
        m1, m2, m3 = _split3(np.float32(-2.0) * cx)
        n1, n2, n3 = _split3(np.float32(-2.0) * cy)
        ones = np.ones_like(csq)
        eps = np.full_like(csq, 1e-30)
        pairs = [
            (s1, ones), (s2, ones), (s3, ones),              # sq_u
            (ones, s1), (ones, s2), (ones, s3),              # sq_v
            (x1, m1), (x1, m2), (x2, m1), (x1, m3), (x3, m1), (x2, m2),
            (y1, n1), (y1, n2), (y2, n1), (y1, n3), (y3, n1), (y2, n2),
            (ones, eps),                                     # ban exact-zero d2
        ]
        # per-community contiguous [U_c || V_c] so the input DMA can be
        # chunked by community and compute starts after the first chunk
        for k, (uu, vv) in enumerate(pairs):
            for c in range(N_COMM):
                UV[b, k, c * 2 * C : c * 2 * C + C] = uu[c * C : (c + 1) * C]
                UV[b, k, c * 2 * C + C : (c + 1) * 2 * C] = vv[c * C : (c + 1) * C]
    return comm_nodes, UV


def _colneg():
    """[128, C+1] uint32: 0x80000000 | col; last column = the AND mask."""
    row = np.empty(C + 1, dtype=np.uint32)
    row[:C] = np.uint32(0x80000000) | np.arange(C, dtype=np.uint32)
    row[C] = np.uint32(0xFFFFFE00)
    return np.broadcast_to(row[None, :], (128, C + 1)).copy()


# --------------------------------------------------------------------------
# device graph
# --------------------------------------------------------------------------
NPSUM = 8      # PSUM banks used for d2 tiles (PE runs ahead of DVE)
NTILES = N_COMM * NTILE


def build_nc():
    """Raw-bass graph (no TileContext: this toolchain's walrus rejects the
    multi-wait drain Tile emits at context exit — each instruction may carry
    at most ~2 sync waits, so all waits here are standalone wait_ge's)."""
    # Race detection off: CoreSim's raw-mode detector does not model
    # same-engine program order (hardware DRAINs serialize DVE ops), and
    # flags the safe DVE write->read chains below.
    nc = bass.Bass(detect_race_conditions=False)
    uv_in = nc.declare_dram_parameter(
        "uv_in", [KDIM, 2 * N_COMM * C], mybir.dt.bfloat16, isOutput=False
    )
    colneg = nc.declare_dram_parameter(
        "colneg", [128, C + 1], mybir.dt.uint32, isOutput=False
    )
    out = nc.declare_dram_parameter(
        "out", [NTILES, 128, 16], mybir.dt.float32, isOutput=True
    )

    from contextlib import ExitStack

    with ExitStack() as st:
        # one completion semaphore per uv chunk: the chunk DMAs spread
        # across parallel HW queues, so a shared counter's increments
        # interleave and would release the PE before a chunk has landed
        dma_in = [st.enter_context(nc.semaphore(f"dma_in{c}")) for c in range(N_COMM)]
        dma_cn = st.enter_context(nc.semaphore("dma_cn"))
        pe_sem = st.enter_context(nc.semaphore("pe_sem"))
        psum_free = st.enter_context(nc.semaphore("psum_free"))
        dve_done = st.enter_context(nc.semaphore("dve_done"))
        dma_out = st.enter_context(nc.semaphore("dma_out"))
        uv_sb = st.enter_context(
            nc.sbuf_tensor("uv_sb", [KDIM, 2 * N_COMM * C], mybir.dt.bfloat16)
        )
        colneg_sb = st.enter_context(
            nc.sbuf_tensor("colneg_sb", [128, C + 1], mybir.dt.uint32)
        )
        kt = st.enter_context(nc.sbuf_tensor("kt", [128, C], mybir.dt.uint32))
        allcand = st.enter_context(
            nc.sbuf_tensor("allcand", [128, NTILES, 16], mybir.dt.float32)
        )
        ps = st.enter_context(
            nc.psum_tensor("ps", [128, NPSUM, C], mybir.dt.float32)
        )
        with nc.Block() as block:

            @block.sync
            def _(sync):
                # one DMA per community so the PE can start after chunk 0
                for c in range(N_COMM):
                    sync.dma_start(
                        out=uv_sb[:, c * 2 * C : (c + 1) * 2 * C],
                        in_=uv_in[:, c * 2 * C : (c + 1) * 2 * C],
                    ).then_inc(dma_in[c], 16)

            @block.gpsimd
            def _(gpsimd):
                # colneg on the (otherwise idle) gpsimd SWDGE queue, in
                # parallel with the SP-queue uv chunks
                gpsimd.dma_start(out=colneg_sb[:], in_=colneg[:]).then_inc(
                    dma_cn, 16
                )

            @block.tensor
            def _(tensor):
                for g in range(NTILES):
                    c, t = divmod(g, NTILE)
                    if t == 0:
                        tensor.wait_ge(dma_in[c], 16)
                    if g >= NPSUM:
                        tensor.wait_ge(psum_free, g - NPSUM + 1)
                    tensor.matmul(
                        ps[:, g % NPSUM, :],
                        uv_sb[:, c * 2 * C + t * 128 : c * 2 * C + (t + 1) * 128],
                        uv_sb[:, c * 2 * C + C : (c + 1) * 2 * C],
                        start=True,
                        stop=True,
                    ).then_inc(pe_sem, 1)

            @block.vector
            def _(vector):
                vector.wait_ge(dma_cn, 16)
                for g in range(NTILES):
                    vector.wait_ge(pe_sem, g + 1)
                    vector.scalar_tensor_tensor(
                        out=kt[:],
                        in0=ps[:, g % NPSUM, :].bitcast(mybir.dt.uint32),
                        scalar=colneg_sb[:, C : C + 1],
                        in1=colneg_sb[:, 0:C],
                        op0=mybir.AluOpType.bitwise_and,
                        op1=mybir.AluOpType.bitwise_or,
                    ).then_inc(psum_free, 1)
                    # two contiguous half-classes: top-8 of cols 0..255 and
                    # of cols 256..511 (keys embed the absolute column)
                    ktf = kt[:].bitcast(mybir.dt.float32)
                    vector.max(out=allcand[:, g, 0:8], in_=ktf[:, 0 : C // 2])
                    mx = vector.max(out=allcand[:, g, 8:16], in_=ktf[:, C // 2 : C])
                    if g == NTILES - 1:
                        mx.then_inc(dve_done, 1)

            @block.scalar
            def _(scalar):
                scalar.wait_ge(dve_done, 1)
                scalar.dma_start(
                    out=out[:].rearrange("g p j -> p g j"),
                    in_=allcand[:],
                ).then_inc(dma_out, 16)
                scalar.wait_ge(dma_out, 16)

    return nc


# --------------------------------------------------------------------------
# host post-processing
# --------------------------------------------------------------------------
def _postprocess(coords, comm_nodes, cand_all):
    """cand_all: [B, N_COMM*C, 16] f32 candidate keys -> (edge_index, edge_weight)."""
    import jax

    cpu = jax.devices("cpu")[0]

    all_src = np.empty((B, N_COMM * C, K_NN), dtype=np.int64)
    all_tgt = np.empty((B, N_COMM * C, K_NN), dtype=np.int64)
    all_d = np.empty((B, N_COMM * C, K_NN), dtype=np.float32)

    rows_local = np.tile(np.arange(C), N_COMM)          # [NC*C] row within community
    comm_of = np.repeat(np.arange(N_COMM), C)           # [NC*C]
    for b in range(B):
        co = coords[b]
        x, y = co[:, 0], co[:, 1]
        sq = x * x + y * y
        cn = comm_nodes[b]                               # [NC, C]
        cols = cand_all[b].view(np.uint32) & np.uint32(0x1FF)   # [NC*C, 16]
        cols = cols.astype(np.int64)
        u_loc = cn[comm_of, rows_local]                  # [NC*C] global node of row
        v_loc = cn[comm_of[:, None], cols]               # [NC*C,16] global node of cand
        dist = _exact_dist(co, sq, np.broadcast_to(u_loc[:, None], cols.shape), v_loc)
        # diagonal (col == own row) is masked to BIG, like the reference
        dist = np.where(cols == rows_local[:, None], BIG, dist)
        # defensive dedupe: repeated candidate cols (should not happen) -> BIG
        cs = np.sort(cols, axis=1)
        dup_vals = np.where(np.diff(cs, axis=1) == 0, cs[:, 1:], -1)
        if (dup_vals >= 0).any():
            for j in range(dup_vals.shape[1]):
                m = dup_vals[:, j] >= 0
                if m.any():
                    hit = cols[m] == dup_vals[m, j][:, None]
                    first = hit.argmax(axis=1)
                    hit[np.arange(hit.shape[0]), first] = False
                    dsub = dist[m]
                    dsub[hit] = BIG
                    dist[m] = dsub
        # top-8 by (dist asc, col asc) == lax.top_k(-dist) tie semantics
        sel = np.lexsort((cols, dist), axis=-1)[:, :K_NN]
        tgt_b = np.take_along_axis(v_loc, sel, 1)
        d_b = np.take_along_axis(dist, sel, 1)

        # Parity candidates give only 8-deep coverage per class; rows whose
        # true top-8 is concentrated in one parity class can miss members
        # (the diagonal eats a slot; near-tie inversions can too).  Any such
        # miss leaves >=6 selected edges sharing one parity -> recompute
        # those rows exactly (pure reference math over the full community).
        selcols = np.take_along_axis(cols, sel, 1)
        n_lo = (selcols < C // 2).sum(axis=1)
        flagged = np.nonzero(np.minimum(n_lo, K_NN - n_lo) <= 2)[0]
        if flagged.size:
            nodes_f = cn[comm_of[flagged]]                        # [F, C]
            u_f = u_loc[flagged]
            dfull = _exact_dist(
                co, sq, np.broadcast_to(u_f[:, None], nodes_f.shape), nodes_f
            )
            dfull[np.arange(flagged.size), rows_local[flagged]] = BIG
            colr = np.broadcast_to(np.arange(C)[None, :], dfull.shape)
            o2 = np.lexsort((colr, dfull), axis=-1)[:, :K_NN]
            tgt_b[flagged] = np.take_along_axis(nodes_f, o2, 1)
            d_b[flagged] = np.take_along_axis(dfull, o2, 1)

        all_tgt[b] = tgt_b
        all_d[b] = d_b
        all_src[b] = np.broadcast_to(u_loc[:, None], (N_COMM * C, K_NN))

    w_intra = np.float32(10.0) * np.exp(np.float32(-2.0) * all_d)

    # ---- inter-community random edges: replicate the vmapped jax PRNG ----
    import jax.numpy as jnp  # noqa: F401  (jax initialized above)

    with jax.default_device(cpu):
        keys = jax.random.split(jax.random.key(42), B)

        def prng_part(key):
            k_perm, k_a, k_b = jax.random.split(key, 3)
            perms = jax.vmap(lambda k: jax.random.permutation(k, N_COMM))(
                jax.random.split(k_perm, N_INTER)
            )
            i1 = jax.random.randint(k_a, (N_INTER,), 0, C)
            i2 = jax.random.randint(k_b, (N_INTER,), 0, C)
            return perms[:, :2], i1, i2

        perm2, i1a, i2a = jax.tree.map(np.asarray, jax.vmap(prng_part)(keys))

    src_parts, tgt_parts, w_parts = [], [], []
    for b in range(B):
        co = coords[b]
        x, y = co[:, 0], co[:, 1]
        sq = x * x + y * y
        cn = comm_nodes[b]
        n1 = cn[perm2[b][:, 0], i1a[b]]
        n2 = cn[perm2[b][:, 1], i2a[b]]
        d_x = _exact_dist(co, sq, n1, n2)
        d_x = np.where(n1 == n2, BIG, d_x)
        w_x = np.float32(0.05) * np.exp(np.float32(-2.0) * d_x)
        src_parts.append(np.concatenate([all_src[b].reshape(-1), n1]) + b * N)
        tgt_parts.append(np.concatenate([all_tgt[b].reshape(-1), n2]) + b * N)
        w_parts.append(np.concatenate([w_intra[b].reshape(-1), w_x]))

    edge_index = np.stack(
        [np.concatenate(src_parts), np.concatenate(tgt_parts)]
    ).astype(np.int32)
    edge_weight = np.concatenate(w_parts).astype(np.float32)
    return edge_index, edge_weight


# --------------------------------------------------------------------------
# entry point
# --------------------------------------------------------------------------
_NC_CACHE = {}
LAST_RESULT = None  # BassKernelResults of the most recent run (for profiling)


def kernel(coords, node_masks):
    global LAST_RESULT
    coords = np.asarray(coords, dtype=np.float32)
    comm_nodes, UV = _host_prep(coords)
    cn_const = _colneg()

    if "nc" not in _NC_CACHE:
        _NC_CACHE["nc"] = build_nc()
    nc = _NC_CACHE["nc"]

    in_maps = [
        {"uv_in": UV[b], "colneg": cn_const} for b in range(B)
    ]
    trace = os.environ.get("KERNEL_TRACE", "0") == "1"
    res = run_bass_kernel_spmd(
        nc, in_maps, core_ids=list(range(B)), trace=trace
    )
    LAST_RESULT = res
    cand_all = np.stack(
        [res.results[b]["out"].reshape(N_COMM * C, 16) for b in range(B)]
    )
    return _postprocess(coords, comm_nodes, cand_all)


if __name__ == "__main__":
    coords = np.random.default_rng(0).standard_normal((B, N, 2)).astype(np.float32)
    ei, ew = kernel(coords, np.ones((B, N), dtype=bool))
    print("edge_index", ei.shape, ei.dtype, "edge_weight", ew.shape, ew.dtype)


# revision 37
# speedup vs baseline: 1.2482x; 1.2482x over previous
"""Trainium2 Bass kernel for nn_A100GraphBuilder (per-sample community kNN graphs).

Strategy (pure data parallelism, one sample per NeuronCore, B=8 -> 8 cores):
  host prep : per sample, sort nodes by x into 8 communities of 512 (argsort),
              build the K=5 matmul operands that evaluate
                d2[u,v] = sq_u + sq_v - 2*x_u*x_v - 2*y_u*y_v + 1e-30
              as a single PE matmul per [128 x 512] tile.
  device    : per community (512x512 block, 4 row tiles):
                PE   : K=5 fp32 matmul -> d2 tile in PSUM
                DVE  : pack keys  K = (bits(d2) & ~0x1FF) | 0x80000000 | col
                       (negated d2 quantized to 14 mantissa bits with the
                        column index embedded in the low 9 bits -> unique keys,
                        f32-descending order == (d2 asc, col asc))
                DVE  : MAX8 -> top-8 keys; MATCH_REPLACE knocks them out;
                       MAX8 again -> candidates 9..16
              out: [4096, 16] candidate keys per core.
  host post : decode candidate columns, drop the diagonal, re-rank the 16
              candidates with bitwise-exact reference distances (the XLA CPU
              recipe: fma(y*y', f32(x*x'))), take top-8 with (dist, col)
              ties -> exact lax.top_k semantics; weights = 10*exp(-2*d).
              Random inter-community edges replicated with the vmapped jax
              PRNG on CPU.  Concatenate to the reference edge layout.
"""

import os
import sys

import numpy as np

for _p in ("/opt/trn_rl_repo", "/root/.axon_site/_ro/trn_rl_repo"):
    if os.path.isdir(_p) and _p not in sys.path:
        sys.path.append(_p)

import ml_dtypes

import concourse.bass as bass
import concourse.mybir as mybir
from concourse.bass_utils import run_bass_kernel_spmd

B = 8
N = 4096
N_COMM = 8
C = 512
K_NN = 8
N_INTER = 8
BIG = np.float32(1.0e6)
KDIM = 19         # matmul contraction rows (3-way bf16 split, see _host_prep)
NTILE = C // 128  # row tiles per community


# --------------------------------------------------------------------------
# host: exact reference distance recipe (bitwise-identical to XLA CPU)
# --------------------------------------------------------------------------
def _exact_dist(co, sq, u, v):
    """dist[u,v] exactly as the CPU reference computes it (f32 bitwise)."""
    t32 = np.float32(co[u, 0] * co[v, 0])
    g = (
        co[u, 1].astype(np.float64) * co[v, 1].astype(np.float64)
        + t32.astype(np.float64)
    ).astype(np.float32)
    d2 = np.maximum((sq[u] + sq[v]) - np.float32(2.0) * g, np.float32(1e-12))
    return np.sqrt(d2)


def _split3(v):
    """3-way bf16 split: v ~= a + b + c with ~24 mantissa bits total."""
    a = v.astype(ml_dtypes.bfloat16).astype(np.float32)
    b = (v - a).astype(ml_dtypes.bfloat16).astype(np.float32)
    c = (v - a - b).astype(ml_dtypes.bfloat16).astype(np.float32)
    return a, b, c


def _host_prep(coords):
    """comm_nodes [B,NC,C] and packed bf16 matmul operands UV [B,19,2*NC*C].

    d2 = sq_u + sq_v - 2 x_u x_v - 2 y_u y_v + 1e-30 evaluated as a K=19
    bf16 matmul: each f32 factor is split into 3 bf16 terms and the
    significant cross-products are accumulated in fp32 PSUM (error ~1e-5,
    well inside the candidate scheme's tolerance).  Columns 0..NC*C hold
    the stationary operand rows, columns NC*C..2*NC*C the moving rows.
    """
    comm_nodes = np.empty((B, N_COMM, C), dtype=np.int64)
    UV = np.zeros((B, KDIM, 2 * N_COMM * C), dtype=ml_dtypes.bfloat16)
    for b in range(B):
        x = coords[b, :, 0]
        y = coords[b, :, 1]
        sq = x * x + y * y
        order = np.argsort(x, kind="stable")
        cn = np.sort(order.reshape(N_COMM, C), axis=1)
        comm_nodes[b] = cn
        flat = cn.reshape(-1)
        cx, cy, csq = x[flat], y[flat], sq[flat]
        s1, s2, s3 = _split3(csq)
        x1, x2, x3 = _split3(cx)
        y1, y2, y3 = _split3(cy)
        m1, m2, m3 = _split3(np.float32(-2.0) * cx)
        n1, n2, n3 = _split3(np.float32(-2.0) * cy)
        ones = np.ones_like(csq)
        eps = np.full_like(csq, 1e-30)
        pairs = [
            (s1, ones), (s2, ones), (s3, ones),              # sq_u
            (ones, s1), (ones, s2), (ones, s3),              # sq_v
            (x1, m1), (x1, m2), (x2, m1), (x1, m3), (x3, m1), (x2, m2),
            (y1, n1), (y1, n2), (y2, n1), (y1, n3), (y3, n1), (y2, n2),
            (ones, eps),                                     # ban exact-zero d2
        ]
        # per-community contiguous [U_c || V_c] so the input DMA can be
        # chunked by community and compute starts after the first chunk
        for k, (uu, vv) in enumerate(pairs):
            for c in range(N_COMM):
                UV[b, k, c * 2 * C : c * 2 * C + C] = uu[c * C : (c + 1) * C]
                UV[b, k, c * 2 * C + C : (c + 1) * 2 * C] = vv[c * C : (c + 1) * C]
    return comm_nodes, UV


def _colneg():
    """[128, C+1] uint32: 0x80000000 | col; last column = the AND mask."""
    row = np.empty(C + 1, dtype=np.uint32)
    row[:C] = np.uint32(0x80000000) | np.arange(C, dtype=np.uint32)
    row[C] = np.uint32(0xFFFFFE00)
    return np.broadcast_to(row[None, :], (128, C + 1)).copy()


# --------------------------------------------------------------------------
# device graph
# --------------------------------------------------------------------------
NPSUM = 8      # PSUM banks used for d2 tiles (PE runs ahead of DVE)
NTILES = N_COMM * NTILE


def build_nc():
    """Raw-bass graph (no TileContext: this toolchain's walrus rejects the
    multi-wait drain Tile emits at context exit — each instruction may carry
    at most ~2 sync waits, so all waits here are standalone wait_ge's)."""
    # Race detection off: CoreSim's raw-mode detector does not model
    # same-engine program order (hardware DRAINs serialize DVE ops), and
    # flags the safe DVE write->read chains below.
    nc = bass.Bass(detect_race_conditions=False)
    uv_in = nc.declare_dram_parameter(
        "uv_in", [KDIM, 2 * N_COMM * C], mybir.dt.bfloat16, isOutput=False
    )
    colneg = nc.declare_dram_parameter(
        "colneg", [128, C + 1], mybir.dt.uint32, isOutput=False
    )
    # p-major dump of allcand: a contiguous [128, NTILES*16] DMA (a
    # g-major layout would need a transposing access pattern -> 64-byte
    # strided AXI writes, ~7x slower); host reorders
    out = nc.declare_dram_parameter(
        "out", [128, NTILES * 16], mybir.dt.float32, isOutput=True
    )

    from contextlib import ExitStack

    with ExitStack() as st:
        # one completion semaphore per uv chunk: the chunk DMAs spread
        # across parallel HW queues, so a shared counter's increments
        # interleave and would release the PE before a chunk has landed
        dma_in = [st.enter_context(nc.semaphore(f"dma_in{c}")) for c in range(N_COMM)]
        dma_cn = st.enter_context(nc.semaphore("dma_cn"))
        pe_sem = st.enter_context(nc.semaphore("pe_sem"))
        psum_free = st.enter_context(nc.semaphore("psum_free"))
        dve_done = st.enter_context(nc.semaphore("dve_done"))
        dma_out = st.enter_context(nc.semaphore("dma_out"))
        uv_sb = st.enter_context(
            nc.sbuf_tensor("uv_sb", [KDIM, 2 * N_COMM * C], mybir.dt.bfloat16)
        )
        colneg_sb = st.enter_context(
            nc.sbuf_tensor("colneg_sb", [128, C + 1], mybir.dt.uint32)
        )
        kt = st.enter_context(nc.sbuf_tensor("kt", [128, C], mybir.dt.uint32))
        allcand = st.enter_context(
            nc.sbuf_tensor("allcand", [128, NTILES, 16], mybir.dt.float32)
        )
        ps = st.enter_context(
            nc.psum_tensor("ps", [128, NPSUM, C], mybir.dt.float32)
        )
        with nc.Block() as block:

            @block.sync
            def _(sync):
                # uv chunk 0 first (PE starts on it), then colneg (needed by
                # the first DVE pack), then the remaining chunks
                def uv_chunk(c):
                    sync.dma_start(
                        out=uv_sb[:, c * 2 * C : (c + 1) * 2 * C],
                        in_=uv_in[:, c * 2 * C : (c + 1) * 2 * C],
                    ).then_inc(dma_in[c], 16)

                uv_chunk(0)
                sync.dma_start(out=colneg_sb[:], in_=colneg[:]).then_inc(
                    dma_cn, 16
                )
                for c in range(1, N_COMM):
                    uv_chunk(c)

            @block.tensor
            def _(tensor):
                for g in range(NTILES):
                    c, t = divmod(g, NTILE)
                    if t == 0:
                        tensor.wait_ge(dma_in[c], 16)
                    if g >= NPSUM:
                        tensor.wait_ge(psum_free, g - NPSUM + 1)
                    tensor.matmul(
                        ps[:, g % NPSUM, :],
                        uv_sb[:, c * 2 * C + t * 128 : c * 2 * C + (t + 1) * 128],
                        uv_sb[:, c * 2 * C + C : (c + 1) * 2 * C],
                        start=True,
                        stop=True,
                    ).then_inc(pe_sem, 1)

            @block.vector
            def _(vector):
                vector.wait_ge(dma_cn, 16)
                for g in range(NTILES):
                    vector.wait_ge(pe_sem, g + 1)
                    vector.scalar_tensor_tensor(
                        out=kt[:],
                        in0=ps[:, g % NPSUM, :].bitcast(mybir.dt.uint32),
                        scalar=colneg_sb[:, C : C + 1],
                        in1=colneg_sb[:, 0:C],
                        op0=mybir.AluOpType.bitwise_and,
                        op1=mybir.AluOpType.bitwise_or,
                    ).then_inc(psum_free, 1)
                    # two contiguous half-classes: top-8 of cols 0..255 and
                    # of cols 256..511 (keys embed the absolute column)
                    ktf = kt[:].bitcast(mybir.dt.float32)
                    vector.max(out=allcand[:, g, 0:8], in_=ktf[:, 0 : C // 2])
                    mx = vector.max(out=allcand[:, g, 8:16], in_=ktf[:, C // 2 : C])
                    if g == NTILES - 1:
                        mx.then_inc(dve_done, 1)

            @block.scalar
            def _(scalar):
                scalar.wait_ge(dve_done, 1)
                scalar.dma_start(out=out[:], in_=allcand[:]).then_inc(
                    dma_out, 16
                )
                scalar.wait_ge(dma_out, 16)

    return nc


# --------------------------------------------------------------------------
# host post-processing
# --------------------------------------------------------------------------
def _postprocess(coords, comm_nodes, cand_all):
    """cand_all: [B, N_COMM*C, 16] f32 candidate keys -> (edge_index, edge_weight)."""
    import jax

    cpu = jax.devices("cpu")[0]

    all_src = np.empty((B, N_COMM * C, K_NN), dtype=np.int64)
    all_tgt = np.empty((B, N_COMM * C, K_NN), dtype=np.int64)
    all_d = np.empty((B, N_COMM * C, K_NN), dtype=np.float32)

    rows_local = np.tile(np.arange(C), N_COMM)          # [NC*C] row within community
    comm_of = np.repeat(np.arange(N_COMM), C)           # [NC*C]
    for b in range(B):
        co = coords[b]
        x, y = co[:, 0], co[:, 1]
        sq = x * x + y * y
        cn = comm_nodes[b]                               # [NC, C]
        cols = cand_all[b].view(np.uint32) & np.uint32(0x1FF)   # [NC*C, 16]
        cols = cols.astype(np.int64)
        u_loc = cn[comm_of, rows_local]                  # [NC*C] global node of row
        v_loc = cn[comm_of[:, None], cols]               # [NC*C,16] global node of cand
        dist = _exact_dist(co, sq, np.broadcast_to(u_loc[:, None], cols.shape), v_loc)
        # diagonal (col == own row) is masked to BIG, like the reference
        dist = np.where(cols == rows_local[:, None], BIG, dist)
        # defensive dedupe: repeated candidate cols (should not happen) -> BIG
        cs = np.sort(cols, axis=1)
        dup_vals = np.where(np.diff(cs, axis=1) == 0, cs[:, 1:], -1)
        if (dup_vals >= 0).any():
            for j in range(dup_vals.shape[1]):
                m = dup_vals[:, j] >= 0
                if m.any():
                    hit = cols[m] == dup_vals[m, j][:, None]
                    first = hit.argmax(axis=1)
                    hit[np.arange(hit.shape[0]), first] = False
                    dsub = dist[m]
                    dsub[hit] = BIG
                    dist[m] = dsub
        # top-8 by (dist asc, col asc) == lax.top_k(-dist) tie semantics
        sel = np.lexsort((cols, dist), axis=-1)[:, :K_NN]
        tgt_b = np.take_along_axis(v_loc, sel, 1)
        d_b = np.take_along_axis(dist, sel, 1)

        # Parity candidates give only 8-deep coverage per class; rows whose
        # true top-8 is concentrated in one parity class can miss members
        # (the diagonal eats a slot; near-tie inversions can too).  Any such
        # miss leaves >=6 selected edges sharing one parity -> recompute
        # those rows exactly (pure reference math over the full community).
        selcols = np.take_along_axis(cols, sel, 1)
        n_lo = (selcols < C // 2).sum(axis=1)
        flagged = np.nonzero(np.minimum(n_lo, K_NN - n_lo) <= 2)[0]
        if flagged.size:
            nodes_f = cn[comm_of[flagged]]                        # [F, C]
            u_f = u_loc[flagged]
            dfull = _exact_dist(
                co, sq, np.broadcast_to(u_f[:, None], nodes_f.shape), nodes_f
            )
            dfull[np.arange(flagged.size), rows_local[flagged]] = BIG
            colr = np.broadcast_to(np.arange(C)[None, :], dfull.shape)
            o2 = np.lexsort((colr, dfull), axis=-1)[:, :K_NN]
            tgt_b[flagged] = np.take_along_axis(nodes_f, o2, 1)
            d_b[flagged] = np.take_along_axis(dfull, o2, 1)

        all_tgt[b] = tgt_b
        all_d[b] = d_b
        all_src[b] = np.broadcast_to(u_loc[:, None], (N_COMM * C, K_NN))

    w_intra = np.float32(10.0) * np.exp(np.float32(-2.0) * all_d)

    # ---- inter-community random edges: replicate the vmapped jax PRNG ----
    import jax.numpy as jnp  # noqa: F401  (jax initialized above)

    with jax.default_device(cpu):
        keys = jax.random.split(jax.random.key(42), B)

        def prng_part(key):
            k_perm, k_a, k_b = jax.random.split(key, 3)
            perms = jax.vmap(lambda k: jax.random.permutation(k, N_COMM))(
                jax.random.split(k_perm, N_INTER)
            )
            i1 = jax.random.randint(k_a, (N_INTER,), 0, C)
            i2 = jax.random.randint(k_b, (N_INTER,), 0, C)
            return perms[:, :2], i1, i2

        perm2, i1a, i2a = jax.tree.map(np.asarray, jax.vmap(prng_part)(keys))

    src_parts, tgt_parts, w_parts = [], [], []
    for b in range(B):
        co = coords[b]
        x, y = co[:, 0], co[:, 1]
        sq = x * x + y * y
        cn = comm_nodes[b]
        n1 = cn[perm2[b][:, 0], i1a[b]]
        n2 = cn[perm2[b][:, 1], i2a[b]]
        d_x = _exact_dist(co, sq, n1, n2)
        d_x = np.where(n1 == n2, BIG, d_x)
        w_x = np.float32(0.05) * np.exp(np.float32(-2.0) * d_x)
        src_parts.append(np.concatenate([all_src[b].reshape(-1), n1]) + b * N)
        tgt_parts.append(np.concatenate([all_tgt[b].reshape(-1), n2]) + b * N)
        w_parts.append(np.concatenate([w_intra[b].reshape(-1), w_x]))

    edge_index = np.stack(
        [np.concatenate(src_parts), np.concatenate(tgt_parts)]
    ).astype(np.int32)
    edge_weight = np.concatenate(w_parts).astype(np.float32)
    return edge_index, edge_weight


# --------------------------------------------------------------------------
# entry point
# --------------------------------------------------------------------------
_NC_CACHE = {}
LAST_RESULT = None  # BassKernelResults of the most recent run (for profiling)


def kernel(coords, node_masks):
    global LAST_RESULT
    coords = np.asarray(coords, dtype=np.float32)
    comm_nodes, UV = _host_prep(coords)
    cn_const = _colneg()

    if "nc" not in _NC_CACHE:
        _NC_CACHE["nc"] = build_nc()
    nc = _NC_CACHE["nc"]

    in_maps = [
        {"uv_in": UV[b], "colneg": cn_const} for b in range(B)
    ]
    trace = os.environ.get("KERNEL_TRACE", "0") == "1"
    res = run_bass_kernel_spmd(
        nc, in_maps, core_ids=list(range(B)), trace=trace
    )
    LAST_RESULT = res
    cand_all = np.stack(
        [
            res.results[b]["out"]
            .reshape(128, NTILES, 16)
            .transpose(1, 0, 2)
            .reshape(N_COMM * C, 16)
            for b in range(B)
        ]
    )
    return _postprocess(coords, comm_nodes, cand_all)


if __name__ == "__main__":
    coords = np.random.default_rng(0).standard_normal((B, N, 2)).astype(np.float32)
    ei, ew = kernel(coords, np.ones((B, N), dtype=bool))
    print("edge_index", ei.shape, ei.dtype, "edge_weight", ew.shape, ew.dtype)
